# revision 1
# baseline (speedup 1.0000x reference)
"""Trainium2 Bass kernel for a bilinear field-interaction layer.

Computation (per example b):
  v[f]   = fields[f, b] @ W + bw                      # shared Dense(D)
  p[i,j] = dot(v[i], fields[j, b])  for i < j          # 780 pairs
  out[b] = p @ Wc + bc                                 # Dense(OUT)

Shapes: fields [40, 16384, 32], W [32, 32], Wc [780, 50] -> out [16384, 50].

Strategy (8 NeuronCores, batch-sharded 2048/core):
  - DMA fields tiles as [128 b, (f,d)]; DVE StreamTranspose (32x32 blocks)
    gives x_t[32q+d, 32f+b'] : d-on-partitions for 4 row-groups (b-blocks q).
  - step1: v^T = W^T x^T via 4 concurrent tile_position=(32q,32q) matmuls,
    W replicated per row-group; +bw on PSUM evacuation.
  - pairwise: one matmul per example: G[j,i] = sum_d x[j,d] v[i,d],
    K=32 M=40 N=40, packed 8-concurrent via tile_position=(32q, 64*c2),
    12/4 example-slots per PSUM bank; evacuated to S[(c2,j), (t,gen,q,slot,i)].
  - final: out^T[o, cols] = sum_i Wc_i[j,o]^T @ S-slices, 40 accumulating
    matmuls per (c2, gen) with 3-dim strided rhs APs; PE-transpose back to
    [b, o] and scatter-DMA rows to batch order.
"""

import sys

sys.path.insert(0, "/opt/trn_rl_repo")

from contextlib import ExitStack

import numpy as np

import concourse.bass as bass
import concourse.tile as tile
from concourse import mybir
from concourse._compat import with_exitstack
from concourse.bass_utils import run_bass_kernel_spmd

F, D, OUT = 40, 32, 50
FD = F * D  # 1280
NPAIR = F * (F - 1) // 2
N_CORES = 8
B_FULL = 16384
BC = B_FULL // N_CORES  # 2048 per core
NT = BC // 128          # 16 b-tiles per core
CHUNK_T = 4             # b-tiles per final-stage chunk
GENS = ((0, 12), (1, 4))  # (gen index, slots per c2); 2*(12+4)=32 = b-block size
SPT = 2560              # S free size per b-tile: genA 4q*480 + genB 4q*160


def _gen_geometry(gen):
    nslot = GENS[gen][1]
    sbase = 0 if gen == 0 else 1920      # S free base within a t-slice
    gq = 40 * nslot                      # S q-stride (480 / 160)
    w0 = 0 if gen == 0 else 24           # first w (example-in-block) of the gen
    return nslot, sbase, gq, w0


@with_exitstack
def build_kernel(ctx: ExitStack, tc: tile.TileContext, out_ext, fields_ext,
                 w_rep_ext, wc_ext, ident_ext, bw_ext, bc_ext, bc_count=BC,
                 bw_is_zero=False):
    nc = tc.nc
    f32 = mybir.dt.float32
    nt = bc_count // 128
    n_chunks = (nt + CHUNK_T - 1) // CHUNK_T

    const = ctx.enter_context(tc.tile_pool(name="const", bufs=1))
    sbuf = ctx.enter_context(tc.tile_pool(name="sbuf", bufs=2))
    spool = ctx.enter_context(tc.tile_pool(name="spool", bufs=2))
    opool = ctx.enter_context(tc.tile_pool(name="opool", bufs=2))
    psum = ctx.enter_context(tc.tile_pool(name="psum", bufs=1, space="PSUM"))

    # ---- constants
    w_rep = const.tile([128, D], f32)
    nc.sync.dma_start(w_rep[:], w_rep_ext[:])
    wc_sb = const.tile([128, F * OUT], f32)
    nc.sync.dma_start(wc_sb[0:104, :], wc_ext[:])
    ident = const.tile([128, OUT], f32)
    nc.sync.dma_start(ident[:], ident_ext[:])
    bw_sb = const.tile([128, 1], f32)
    nc.sync.dma_start(bw_sb[:], bw_ext[:])
    bc_sb = const.tile([128, OUT], f32)
    nc.sync.dma_start(bc_sb[:], bc_ext[:])

    # fields [F, BC, D] viewed as [t, p, f, d]
    fields_r = fields_ext.rearrange("f (t p) d -> t p f d", p=128)
    out_r = out_ext.rearrange("(t p) o -> t p o", p=128)

    # ---- persistent pairwise PSUM banks (gap rows inited once)
    g_ps = [psum.tile([128, 480], f32, name=f"gps{q}") for q in range(4)]
    for q in range(4):
        nc.vector.memset(g_ps[q][32:64, :], 0.0)

    # GPSIMD compute cannot access PSUM; rotate PSUM evacuations DVE/ACT.
    copy_fns = [
        lambda d, s: nc.vector.tensor_copy(d, s),
        lambda d, s: nc.scalar.copy(d, s),
    ]
    cp_idx = [0]

    def copy(dst, src):
        copy_fns[cp_idx[0] % 2](dst, src)
        cp_idx[0] += 1

    for chunk in range(n_chunks):
        t0 = chunk * CHUNK_T
        tn = min(CHUNK_T, nt - t0)
        s_sb = spool.tile([128, tn * SPT], f32, name="s_sb", tag="s_sb")

        for tt in range(tn):
            t = t0 + tt
            # ---- load + transpose
            x_nat = sbuf.tile([128, FD], f32, name="x_nat", tag="x_nat")
            nc.sync.dma_start(x_nat.rearrange("p (f d) -> p f d", d=D), fields_r[t])
            x_t = sbuf.tile([128, FD], f32, name="x_t", tag="x_t")
            nc.vector.transpose(x_t[:], x_nat[:])

            # ---- step1: v^T = W^T @ x^T (+bw)
            v_t = sbuf.tile([128, FD], f32, name="v_t", tag="v_t")
            for s in range(3):
                n0, n1 = s * 512, min(FD, (s + 1) * 512)
                v_ps = psum.tile([128, 512], f32, name="v_ps", tag="v_ps")
                for q in range(4):
                    nc.tensor.matmul(v_ps[32*q:32*q+32, :n1-n0],
                                     w_rep[32*q:32*q+32, :],
                                     x_t[32*q:32*q+32, n0:n1],
                                     start=True, stop=True,
                                     tile_position=(32*q, 32*q))
                if bw_is_zero:
                    copy(v_t[:, n0:n1], v_ps[:, :n1-n0])
                else:
                    nc.vector.tensor_scalar_add(v_t[:, n0:n1], v_ps[:, :n1-n0],
                                                bw_sb[:, 0:1])

            # ---- pairwise
            for gen, nslot in GENS:
                _, sbase, gq, w0 = _gen_geometry(gen)
                for q in range(4):
                    for c2 in range(2):
                        for slot in range(nslot):
                            bp = w0 + c2 * nslot + slot
                            lhsT = x_t[32*q:32*q+32, bp:bp+32*(F-1)+1:32]
                            rhs = v_t[32*q:32*q+32, bp:bp+32*(F-1)+1:32]
                            nc.tensor.matmul(
                                g_ps[q][64*c2:64*c2+F, 40*slot:40*slot+F],
                                lhsT, rhs, start=True, stop=True,
                                tile_position=(32*q, 64*c2))
                for q in range(4):
                    base = tt * SPT + sbase + q * gq
                    copy(s_sb[0:104, base:base+gq], g_ps[q][0:104, 0:gq])

        # ---- final accumulation: acc[c2] cols = genA[t',q,s] then genB[t',q,s]
        ga_n = tn * 48
        gb_n = tn * 16
        accs = []
        for c2 in range(2):
            acc = psum.tile([128, ga_n + gb_n], f32, name=f"acc{c2}",
                            tag=f"acc{c2}")
            accs.append(acc)
            p0 = 64 * c2
            for i in range(F):
                lhsT = wc_sb[p0:p0+F, i*OUT:(i+1)*OUT]
                for gi, (gen, nslot) in enumerate(GENS):
                    _, sbase, gq, _ = _gen_geometry(gen)
                    # 3-dim strided free AP: (t', q, slot) at offset sbase+i
                    rhs = s_sb[p0:p0+F, 0:tn*SPT]
                    rhs = rhs.rearrange("p (t x) -> p t x", t=tn)
                    rhs = rhs[:, :, sbase:sbase+4*gq]
                    rhs = rhs.rearrange("p t (q s e) -> p t q s e",
                                        q=4, s=nslot)[:, :, :, :, i]
                    cb = 0 if gen == 0 else ga_n
                    o_ap = accs[c2][p0:p0+OUT, cb:cb+tn*(48 if gen == 0 else 16)]
                    nc.tensor.matmul(o_ap, lhsT, rhs,
                                     start=(i == 0 and gi == 0),
                                     stop=(i == F - 1 and gi == len(GENS) - 1),
                                     tile_position=(p0, p0),
                                     skip_group_check=True)

        # ---- evacuate acc, transpose to [b, o], add bc, scatter out
        for c2 in range(2):
            p0 = 64 * c2
            a_sb = opool.tile([128, ga_n + gb_n], f32, name=f"a_sb{c2}",
                              tag=f"a_sb{c2}")
            copy(a_sb[p0:p0+OUT, :], accs[c2][p0:p0+OUT, :])
            # transpose blocks: genA in pieces of 96 cols (2 t'), genB one 128/64
            blocks = []
            for tb in range(0, tn, 2):
                w = min(2, tn - tb) * 48
                blocks.append((tb * 48, w, 0, tb))
            blocks.append((ga_n, gb_n, 1, 0))
            for (cb, w, gen, tb) in blocks:
                o_ps = psum.tile([128, OUT], f32, name="o_ps", tag="o_ps")
                nc.tensor.matmul(o_ps[0:w, :], a_sb[p0:p0+OUT, cb:cb+w],
                                 ident[p0:p0+OUT, :], is_transpose=True,
                                 start=True, stop=True,
                                 tile_position=(p0, 0),
                                 skip_group_check=True)
                o_sb = opool.tile([128, OUT], f32, name="o_sb", tag="o_sb")
                nc.vector.tensor_add(o_sb[0:w, :], o_ps[0:w, :], bc_sb[0:w, :])
                # scatter rows to out: per t' segment
                nslot = GENS[gen][1]
                w0 = (0 if gen == 0 else 24) + c2 * nslot
                seg = 4 * nslot
                for k in range(w // seg):
                    t_ = (tb + k) if gen == 0 else k
                    dst = out_r[t0 + t_].rearrange("(q w) o -> q w o", q=4)
                    nc.sync.dma_start(dst[:, w0:w0+nslot, :],
                                      o_sb[k*seg:(k+1)*seg, :])


def _host_prep(W, bw, Wc, bc):
    w_rep = np.tile(np.asarray(W, np.float32), (4, 1))
    iu, ju = np.triu_indices(F, k=1)
    SFull = np.zeros((F, F, OUT), np.float32)
    SFull[iu, ju] = np.asarray(Wc, np.float32)
    wcT = np.ascontiguousarray(np.transpose(SFull, (1, 0, 2))).reshape(F, F * OUT)
    wc_host = np.zeros((104, F * OUT), np.float32)
    wc_host[0:40] = wcT
    wc_host[64:104] = wcT
    ident = np.zeros((128, OUT), np.float32)
    for p in range(OUT):
        ident[p, p] = 1.0
        ident[64 + p, p] = 1.0
    bw_host = np.tile(np.asarray(bw, np.float32).reshape(-1, 1), (4, 1))
    bc_host = np.tile(np.asarray(bc, np.float32).reshape(1, -1), (128, 1))
    return w_rep, wc_host, ident, bw_host, bc_host


_WAIT_CAPS = {}
_WAIT_CAP_DEFAULT = 1


def legalize_waits(nc):
    """Walrus codegen accepts only a limited number of sync-wait commands per
    instruction (1 for matmul's S3_LW path, 2 for most others).  Hoist excess
    waits onto no-op instructions on the same engine immediately before."""
    for fn in nc.m.functions:
        for blk in fn.blocks:
            out = []
            for inst in blk.instructions:
                si = getattr(inst, "sync_info", None)
                waits = list(si.on_wait) if si is not None and si.on_wait else []
                cap = _WAIT_CAPS.get(type(inst).__name__, _WAIT_CAP_DEFAULT)
                if len(waits) > cap:
                    excess, keep = waits[:-cap], waits[-cap:]
                    for ci, w in enumerate(excess):
                        nop = mybir.InstNoOp(
                            name=f"{inst.name}-waitsplit{ci}",
                            sync_info=mybir.SyncInfo(on_wait=[w], on_update=[]),
                            bass_nofuse=True,
                            engine=inst.engine,
                        )
                        out.append(nop)
                    si.on_wait = keep
                out.append(inst)
            blk.instructions[:] = out


def make_nc(bc_count=BC, bw_is_zero=False, legalize=True):
    nc = bass.Bass()
    fields_ext = nc.declare_dram_parameter("fields_c", [F, bc_count, D],
                                           mybir.dt.float32, isOutput=False)
    w_rep_ext = nc.declare_dram_parameter("w_rep", [128, D], mybir.dt.float32,
                                          isOutput=False)
    wc_ext = nc.declare_dram_parameter("wc", [104, F * OUT], mybir.dt.float32,
                                       isOutput=False)
    ident_ext = nc.declare_dram_parameter("ident", [128, OUT], mybir.dt.float32,
                                          isOutput=False)
    bw_ext = nc.declare_dram_parameter("bw_r", [128, 1], mybir.dt.float32,
                                       isOutput=False)
    bc_ext = nc.declare_dram_parameter("bc_r", [128, OUT], mybir.dt.float32,
                                       isOutput=False)
    out_ext = nc.declare_dram_parameter("out", [bc_count, OUT], mybir.dt.float32,
                                        isOutput=True)
    with tile.TileContext(nc) as tc:
        build_kernel(tc, out_ext, fields_ext, w_rep_ext, wc_ext, ident_ext,
                     bw_ext, bc_ext, bc_count=bc_count, bw_is_zero=bw_is_zero)
    if legalize:
        legalize_waits(nc)
    return nc


def kernel(fields, W, bw, Wc, bc):
    fields = np.asarray(fields, np.float32)
    w_rep, wc_host, ident, bw_host, bc_host = _host_prep(W, bw, Wc, bc)
    nc = make_nc(BC, bw_is_zero=bool(np.all(np.asarray(bw) == 0)))
    in_maps = []
    for c in range(N_CORES):
        in_maps.append({
            "fields_c": np.ascontiguousarray(fields[:, c*BC:(c+1)*BC, :]),
            "w_rep": w_rep, "wc": wc_host, "ident": ident,
            "bw_r": bw_host, "bc_r": bc_host,
        })
    res = run_bass_kernel_spmd(nc, in_maps, list(range(N_CORES)))
    outs = [res.results[c]["out"] for c in range(N_CORES)]
    return np.concatenate(outs, axis=0).astype(np.float32)



# revision 13
# speedup vs baseline: 4.2156x; 4.2156x over previous
"""Trainium2 Bass kernel for a bilinear field-interaction layer.

Computation (per example b):
  v[f]   = fields[f, b] @ W + bw                      # shared Dense(D)
  p[i,j] = dot(v[i], fields[j, b])  for i < j          # 780 pairs
  out[b] = p @ Wc + bc                                 # Dense(OUT)

Shapes: fields [40, 16384, 32], W [32, 32], Wc [780, 50] -> out [16384, 50].

Strategy (8 NeuronCores, batch-sharded 2048/core, all-bf16 compute):
  - HOST pre-transposes fields into the on-chip layout x_t[32q+d,
    1280t + 32f + b'] (bf16), so DMA is one contiguous 10KB-run/partition
    load per 4-tile chunk and no on-device transpose exists.
  - step1: v = x W as ONE matmul per 512-col PSUM segment using a
    host-built block-diagonal W4 [128,128] (4 replicas of W on the
    diagonal) -> v_t bf16, same (32i+b') column layout as x_t.
  - pairwise, per example (q, b'): stationary X^T [32d, 40j] (col stride
    32), two N=20 matmuls moving v-even / v-odd i columns (stride 64)
    into PSUM partition rows 0:40 / 64:104.  G[j, i] = v_i . x_j.
  - G banks of 24 examples evacuated (DVE/ACT alternating) to
    S[64a + j, 512k + b] bf16 where i = 2k + a  (i-pairs along K).
  - final: per 512-example chunk, 20 accumulating matmuls
    acc[o, b] += WCP[:, k]^T @ S[:, k-slice]  with K=104 (j rows 40:64
    are PSUM garbage x host-zeroed Wc rows -> 0).  Output stays [o, b];
    host transposes back.  bias adds fused on the PSUM evacuations.
"""

import sys

sys.path.insert(0, "/opt/trn_rl_repo")

from contextlib import ExitStack

import numpy as np
import ml_dtypes

import concourse.bass as bass
import concourse.tile as tile
from concourse import mybir
from concourse._compat import with_exitstack
from concourse.bass_utils import run_bass_kernel_spmd

F, D, OUT = 40, 32, 50
N_CORES = 8
B_FULL = 16384
BC = B_FULL // N_CORES   # 2048 per core
NT = BC // 128           # 16 b-tiles per core
CHUNK_T = 4              # b-tiles per chunk (S/final granularity)
BCH = CHUNK_T * 128      # 512 examples per chunk
TILE_C = F * D           # 1280 bf16 cols per b-tile
KP = F // 2              # 20 i-pairs (i = 2k + a)
NG = 4                   # rotating G PSUM tiles


@with_exitstack
def build_kernel(ctx: ExitStack, tc: tile.TileContext, out_ext, x_ext,
                 w4_ext, wcp_ext, bw_ext, bc_ext, bc_count=BC,
                 bw_is_zero=False, bc_is_zero=False, stages=3):
    nc = tc.nc
    f32 = mybir.dt.float32
    bf16 = mybir.dt.bfloat16
    nt = bc_count // 128
    n_chunks = nt // CHUNK_T

    const = ctx.enter_context(tc.tile_pool(name="const", bufs=1))
    xpool = ctx.enter_context(tc.tile_pool(name="xpool", bufs=2))
    vpool = ctx.enter_context(tc.tile_pool(name="vpool", bufs=2))
    spool = ctx.enter_context(tc.tile_pool(name="spool", bufs=2))
    opool = ctx.enter_context(tc.tile_pool(name="opool", bufs=1))
    psum_v = ctx.enter_context(tc.tile_pool(name="psv", bufs=2, space="PSUM"))
    psum_g = ctx.enter_context(tc.tile_pool(name="psg", bufs=1, space="PSUM"))
    psum_a = ctx.enter_context(tc.tile_pool(name="psa", bufs=2, space="PSUM"))

    # ---- constants
    w4 = const.tile([128, 128], bf16)
    nc.sync.dma_start(w4[:], w4_ext[:])
    wcp = const.tile([128, KP * OUT], bf16)
    nc.sync.dma_start(wcp[:], wcp_ext[:])
    bw_sb = const.tile([128, 1], f32)
    nc.sync.dma_start(bw_sb[:], bw_ext[:])
    bc_sb = const.tile([128, 1], f32)
    nc.sync.dma_start(bc_sb[:], bc_ext[:])

    out_sb = opool.tile([128, bc_count], f32)

    # ---- persistent pairwise PSUM banks; rows 40:64 are read by the S
    # evacuation (as K padding vs zeroed Wc rows) -> must be finite.
    g_ps = [psum_g.tile([128, 480], f32, name=f"gps{i}") for i in range(NG)]
    for i in range(NG):
        nc.vector.memset(g_ps[i][32:64, :], 0.0)

    copy_fns = [
        lambda d, s: nc.vector.tensor_copy(d, s),
        lambda d, s: nc.scalar.copy(d, s),
    ]
    cp_idx = [0]

    def copy(dst, src):
        copy_fns[cp_idx[0] % 2](dst, src)
        cp_idx[0] += 1

    x_chunks = [None] * n_chunks
    s_chunks = [None] * n_chunks
    v_tiles = [None] * nt

    def emit_dma(c):
        x_sb = xpool.tile([128, CHUNK_T * TILE_C], bf16, name="x_sb",
                          tag="x_sb")
        nc.sync.dma_start(x_sb[:], x_ext[:, c * CHUNK_T * TILE_C:
                                         (c + 1) * CHUNK_T * TILE_C])
        x_chunks[c] = x_sb

    def emit_step1(t):
        c, tt = divmod(t, CHUNK_T)
        xt = x_chunks[c][:, tt * TILE_C:(tt + 1) * TILE_C]
        v_t = vpool.tile([128, TILE_C], bf16, name="v_t", tag="v_t")
        v_tiles[t] = v_t
        for n0, n1 in ((0, 512), (512, 1024), (1024, TILE_C)):
            v_ps = psum_v.tile([128, 512], f32, name="v_ps", tag="v_ps")
            nc.tensor.matmul(v_ps[:, :n1 - n0], w4[:], xt[:, n0:n1],
                             start=True, stop=True, tile_position=(0, 0))
            if bw_is_zero:
                copy(v_t[:, n0:n1], v_ps[:, :n1 - n0])
            else:
                nc.vector.tensor_scalar_add(v_t[:, n0:n1], v_ps[:, :n1 - n0],
                                            bw_sb[:, 0:1])

    def emit_pairwise(t):
        if stages < 2:
            return
        c, tt = divmod(t, CHUNK_T)
        xt = x_chunks[c][:, tt * TILE_C:(tt + 1) * TILE_C]
        v_t = v_tiles[t]
        s_sb = s_chunks[c]
        s_r = s_sb[0:104, :].rearrange("p (k b) -> p k b", b=BCH)

        def evac(q, b0, gsz, c0):
            # src bank-q cols [20c0, 20c0+20gsz) slots -> S cols
            # tile*128 + 4b' + q for b' in [b0, b0+gsz); rows 40:64 carry
            # the zero/garbage K-pad vs zeroed Wc rows.
            src = g_ps[q][0:104, 20 * c0:20 * (c0 + gsz)]
            src = src.rearrange("p (m k) -> p k m", k=KP)
            off = tt * 128 + 4 * b0 + q
            copy(s_r[:, :, off:off + 4 * (gsz - 1) + 1:4], src)

        # generations: b' 0:12 -> bank cols 0:240, 12:24 -> 240:480,
        # 24:32 -> 0:160 (after gen-A evacuation).  Per-q PSUM banks keep
        # concurrent row-tiles out of each other's banks.
        for e in range(128):
            q, bp = e % 4, e // 4
            gen = bp // 12
            col = 20 * (bp % 12) + (240 if gen == 1 else 0)
            lhsT = xt[32 * q:32 * q + 32, bp:bp + 32 * (F - 1) + 1:32]
            rhsE = v_t[32 * q:32 * q + 32, bp:bp + 64 * (KP - 1) + 1:64]
            rhsO = v_t[32 * q:32 * q + 32,
                       32 + bp:32 + bp + 64 * (KP - 1) + 1:64]
            gt = g_ps[q]
            nc.tensor.matmul(gt[0:F, col:col + KP], lhsT, rhsE,
                             start=True, stop=True,
                             tile_position=(32 * q, 0))
            nc.tensor.matmul(gt[64:64 + F, col:col + KP], lhsT, rhsO,
                             start=True, stop=True,
                             tile_position=(32 * q, 64))
            if e == 51:
                for q2 in range(4):
                    evac(q2, 0, 12, 0)
            elif e == 99:
                for q2 in range(4):
                    evac(q2, 12, 12, 12)
        for q2 in range(4):
            evac(q2, 24, 8, 0)

    def emit_final(c):
        ob = out_sb[0:OUT, c * BCH:(c + 1) * BCH]
        if stages < 3:
            nc.vector.memset(ob, 0.0)
            nc.sync.dma_start(out_ext[:, c * BCH:(c + 1) * BCH], ob)
            return
        s_sb = s_chunks[c]
        acc = psum_a.tile([128, BCH], f32, name="acc", tag="acc")
        for k in range(KP):
            nc.tensor.matmul(acc[0:OUT, :], wcp[0:104, k * OUT:(k + 1) * OUT],
                             s_sb[0:104, k * BCH:(k + 1) * BCH],
                             start=(k == 0), stop=(k == KP - 1),
                             tile_position=(0, 0), skip_group_check=True)
        if bc_is_zero:
            copy(ob, acc[0:OUT, :])
        else:
            nc.vector.tensor_scalar_add(ob, acc[0:OUT, :], bc_sb[0:OUT, 0:1])
        nc.sync.dma_start(out_ext[:, c * BCH:(c + 1) * BCH], ob)

    # ---- software pipeline: step1 runs one tile ahead of pairwise so the
    # v evacuation (DVE/ACT) hides behind the previous tile's PE work.
    emit_dma(0)
    s_chunks[0] = spool.tile([128, KP * BCH], bf16, name="s_sb", tag="s_sb")
    emit_step1(0)
    for t in range(nt):
        if t % CHUNK_T == 0 and t // CHUNK_T + 1 < n_chunks:
            emit_dma(t // CHUNK_T + 1)
        if t + 1 < nt:
            if (t + 1) % CHUNK_T == 0:
                s_chunks[t // CHUNK_T + 1] = spool.tile(
                    [128, KP * BCH], bf16, name="s_sb", tag="s_sb")
            emit_step1(t + 1)
        emit_pairwise(t)
        if t % CHUNK_T == CHUNK_T - 1:
            emit_final(t // CHUNK_T)


def _host_prep(fields, W, bw, Wc, bc):
    # x_t[core][32q+d, 1280t + 32f + b'] = fields[f, 2048c + 128t + 4b' + q, d]
    # (in-tile example index 4b'+q == the kernel's pairwise slot order, so
    # S columns / output columns come out in natural batch order)
    x = np.asarray(fields, np.float32).reshape(F, N_CORES, NT, 32, 4, D)
    x_t = np.transpose(x, (1, 4, 5, 2, 0, 3))  # [c, q, d, t, f, b']
    x_t = np.ascontiguousarray(x_t).reshape(N_CORES, 128, NT * TILE_C)
    x_t = x_t.astype(ml_dtypes.bfloat16)

    w4 = np.zeros((128, 128), np.float32)
    for a in range(4):
        w4[32 * a:32 * a + 32, 32 * a:32 * a + 32] = np.asarray(W, np.float32)
    w4 = w4.astype(ml_dtypes.bfloat16)

    iu, ju = np.triu_indices(F, k=1)
    wc_full = np.zeros((F, F, OUT), np.float32)
    wc_full[iu, ju] = np.asarray(Wc, np.float32)          # [i, j, o]
    wcp = np.zeros((128, KP * OUT), np.float32)
    for a in range(2):
        for k in range(KP):
            # rows 64a + j  <-  Wc[i=2k+a, j, :]
            wcp[64 * a:64 * a + F, OUT * k:OUT * (k + 1)] = wc_full[2 * k + a]
    wcp = wcp.astype(ml_dtypes.bfloat16)

    bw_host = np.tile(np.asarray(bw, np.float32).reshape(-1, 1), (4, 1))
    bc_host = np.zeros((128, 1), np.float32)
    bc_host[0:OUT, 0] = np.asarray(bc, np.float32)
    return x_t, w4, wcp, bw_host, bc_host


_WAIT_CAPS = {}
_WAIT_CAP_DEFAULT = 1


def legalize_waits(nc):
    """Walrus codegen accepts only a limited number of sync-wait commands per
    instruction (1 for matmul's S3_LW path, 2 for most others).  Hoist excess
    waits onto no-op instructions on the same engine immediately before."""
    for fn in nc.m.functions:
        for blk in fn.blocks:
            out = []
            for inst in blk.instructions:
                si = getattr(inst, "sync_info", None)
                waits = list(si.on_wait) if si is not None and si.on_wait else []
                cap = _WAIT_CAPS.get(type(inst).__name__, _WAIT_CAP_DEFAULT)
                if len(waits) > cap:
                    excess, keep = waits[:-cap], waits[-cap:]
                    for ci, w in enumerate(excess):
                        nop = mybir.InstNoOp(
                            name=f"{inst.name}-waitsplit{ci}",
                            sync_info=mybir.SyncInfo(on_wait=[w], on_update=[]),
                            bass_nofuse=True,
                            engine=inst.engine,
                        )
                        out.append(nop)
                    si.on_wait = keep
                out.append(inst)
            blk.instructions[:] = out
    return nc


def make_nc(bc_count=BC, bw_is_zero=False, bc_is_zero=False, legalize=True,
            stages=3):
    nc = bass.Bass()
    bf16 = mybir.dt.bfloat16
    x_ext = nc.declare_dram_parameter("x_t", [128, (bc_count // 128) * TILE_C],
                                      bf16, isOutput=False)
    w4_ext = nc.declare_dram_parameter("w4", [128, 128], bf16, isOutput=False)
    wcp_ext = nc.declare_dram_parameter("wcp", [128, KP * OUT], bf16,
                                        isOutput=False)
    bw_ext = nc.declare_dram_parameter("bw_r", [128, 1], mybir.dt.float32,
                                       isOutput=False)
    bc_ext = nc.declare_dram_parameter("bc_r", [128, 1], mybir.dt.float32,
                                       isOutput=False)
    out_ext = nc.declare_dram_parameter("out_t", [OUT, bc_count],
                                        mybir.dt.float32, isOutput=True)
    with tile.TileContext(nc) as tc:
        build_kernel(tc, out_ext, x_ext, w4_ext, wcp_ext, bw_ext, bc_ext,
                     bc_count=bc_count, bw_is_zero=bw_is_zero,
                     bc_is_zero=bc_is_zero, stages=stages)
    if legalize:
        legalize_waits(nc)
    return nc


def kernel(fields, W, bw, Wc, bc):
    x_t, w4, wcp, bw_host, bc_host = _host_prep(fields, W, bw, Wc, bc)
    nc = make_nc(BC, bw_is_zero=bool(np.all(np.asarray(bw) == 0)),
                 bc_is_zero=bool(np.all(np.asarray(bc) == 0)))
    in_maps = []
    for c in range(N_CORES):
        in_maps.append({
            "x_t": np.ascontiguousarray(x_t[c]),
            "w4": w4, "wcp": wcp, "bw_r": bw_host, "bc_r": bc_host,
        })
    res = run_bass_kernel_spmd(nc, in_maps, list(range(N_CORES)))
    outs = [res.results[c]["out_t"] for c in range(N_CORES)]  # [50, 2048] each
    full = np.concatenate(outs, axis=1)                        # [50, 16384]
    return np.ascontiguousarray(full.T).astype(np.float32)


# revision 14
# speedup vs baseline: 4.2663x; 1.0120x over previous
"""Trainium2 Bass kernel for a bilinear field-interaction layer.

Computation (per example b):
  v[f]   = fields[f, b] @ W + bw                      # shared Dense(D)
  p[i,j] = dot(v[i], fields[j, b])  for i < j          # 780 pairs
  out[b] = p @ Wc + bc                                 # Dense(OUT)

Shapes: fields [40, 16384, 32], W [32, 32], Wc [780, 50] -> out [16384, 50].

Strategy (8 NeuronCores, batch-sharded 2048/core, all-bf16 compute):
  - HOST pre-transposes fields into the on-chip layout x_t[32q+d,
    1280t + 32f + b'] (bf16), so DMA is one contiguous 10KB-run/partition
    load per 4-tile chunk and no on-device transpose exists.
  - step1: v = x W as ONE matmul per 512-col PSUM segment using a
    host-built block-diagonal W4 [128,128] (4 replicas of W on the
    diagonal) -> v_t bf16, same (32i+b') column layout as x_t.
  - pairwise, per example (q, b'): stationary X^T [32d, 40j] (col stride
    32), two N=20 matmuls moving v-even / v-odd i columns (stride 64)
    into PSUM partition rows 0:40 / 64:104.  G[j, i] = v_i . x_j.
  - G banks of 24 examples evacuated (DVE/ACT alternating) to
    S[64a + j, 512k + b] bf16 where i = 2k + a  (i-pairs along K).
  - final: per 512-example chunk, 20 accumulating matmuls
    acc[o, b] += WCP[:, k]^T @ S[:, k-slice]  with K=104 (j rows 40:64
    are PSUM garbage x host-zeroed Wc rows -> 0).  Output stays [o, b];
    host transposes back.  bias adds fused on the PSUM evacuations.
"""

import sys

sys.path.insert(0, "/opt/trn_rl_repo")

from contextlib import ExitStack

import numpy as np
import ml_dtypes

import concourse.bass as bass
import concourse.tile as tile
from concourse import mybir
from concourse._compat import with_exitstack
from concourse.bass_utils import run_bass_kernel_spmd

F, D, OUT = 40, 32, 50
N_CORES = 8
B_FULL = 16384
BC = B_FULL // N_CORES   # 2048 per core
NT = BC // 128           # 16 b-tiles per core
CHUNK_T = 4              # b-tiles per chunk (S/final granularity)
BCH = CHUNK_T * 128      # 512 examples per chunk
TILE_C = F * D           # 1280 bf16 cols per b-tile
KP = F // 2              # 20 i-pairs (i = 2k + a)
NG = 4                   # rotating G PSUM tiles


@with_exitstack
def build_kernel(ctx: ExitStack, tc: tile.TileContext, out_ext, x_ext,
                 w4_ext, wcp_ext, bw_ext, bc_ext, bc_count=BC,
                 bw_is_zero=False, bc_is_zero=False, stages=3):
    nc = tc.nc
    f32 = mybir.dt.float32
    bf16 = mybir.dt.bfloat16
    nt = bc_count // 128
    n_chunks = nt // CHUNK_T

    const = ctx.enter_context(tc.tile_pool(name="const", bufs=1))
    xpool = ctx.enter_context(tc.tile_pool(name="xpool", bufs=2))
    vpool = ctx.enter_context(tc.tile_pool(name="vpool", bufs=2))
    spool = ctx.enter_context(tc.tile_pool(name="spool", bufs=2))
    opool = ctx.enter_context(tc.tile_pool(name="opool", bufs=1))
    psum_v = ctx.enter_context(tc.tile_pool(name="psv", bufs=2, space="PSUM"))
    psum_g = ctx.enter_context(tc.tile_pool(name="psg", bufs=1, space="PSUM"))
    psum_a = ctx.enter_context(tc.tile_pool(name="psa", bufs=2, space="PSUM"))

    # ---- constants
    w4 = const.tile([128, 128], bf16)
    nc.sync.dma_start(w4[:], w4_ext[:])
    wcp = const.tile([128, KP * OUT], bf16)
    nc.sync.dma_start(wcp[:], wcp_ext[:])
    bw_sb = const.tile([128, 1], f32)
    nc.sync.dma_start(bw_sb[:], bw_ext[:])
    bc_sb = const.tile([128, 1], f32)
    nc.sync.dma_start(bc_sb[:], bc_ext[:])

    out_sb = opool.tile([128, bc_count], f32)

    # ---- persistent pairwise PSUM banks; rows 40:64 are read by the S
    # evacuation (as K padding vs zeroed Wc rows) -> must be finite.
    g_ps = [psum_g.tile([128, 480], f32, name=f"gps{i}") for i in range(NG)]
    for i in range(NG):
        nc.vector.memset(g_ps[i][32:64, :], 0.0)

    copy_fns = [
        lambda d, s: nc.vector.tensor_copy(d, s),
        lambda d, s: nc.scalar.copy(d, s),
    ]
    cp_idx = [0]

    def copy(dst, src):
        copy_fns[cp_idx[0] % 2](dst, src)
        cp_idx[0] += 1

    x_chunks = [None] * n_chunks
    s_chunks = [None] * n_chunks
    v_tiles = [None] * nt

    def emit_dma(c):
        x_sb = xpool.tile([128, CHUNK_T * TILE_C], bf16, name="x_sb",
                          tag="x_sb")
        nc.sync.dma_start(x_sb[:], x_ext[:, c * CHUNK_T * TILE_C:
                                         (c + 1) * CHUNK_T * TILE_C])
        x_chunks[c] = x_sb

    def emit_step1(t):
        c, tt = divmod(t, CHUNK_T)
        xt = x_chunks[c][:, tt * TILE_C:(tt + 1) * TILE_C]
        v_t = vpool.tile([128, TILE_C], bf16, name="v_t", tag="v_t")
        v_tiles[t] = v_t
        for n0, n1 in ((0, 512), (512, 1024), (1024, TILE_C)):
            v_ps = psum_v.tile([128, 512], f32, name="v_ps", tag="v_ps")
            nc.tensor.matmul(v_ps[:, :n1 - n0], w4[:], xt[:, n0:n1],
                             start=True, stop=True, tile_position=(0, 0))
            if bw_is_zero:
                copy(v_t[:, n0:n1], v_ps[:, :n1 - n0])
            else:
                nc.vector.tensor_scalar_add(v_t[:, n0:n1], v_ps[:, :n1 - n0],
                                            bw_sb[:, 0:1])

    def emit_pairwise(t):
        if stages < 2:
            return
        c, tt = divmod(t, CHUNK_T)
        xt = x_chunks[c][:, tt * TILE_C:(tt + 1) * TILE_C]
        v_t = v_tiles[t]
        s_sb = s_chunks[c]
        s_r = s_sb[0:104, :].rearrange("p (k b) -> p k b", b=BCH)

        def evac(q, b0, gsz, c0):
            # src bank-q cols [20c0, 20c0+20gsz) slots -> S cols
            # tile*128 + 4b' + q for b' in [b0, b0+gsz); rows 40:64 carry
            # the zero/garbage K-pad vs zeroed Wc rows.
            src = g_ps[q][0:104, 20 * c0:20 * (c0 + gsz)]
            src = src.rearrange("p (m k) -> p k m", k=KP)
            off = tt * 128 + 4 * b0 + q
            copy(s_r[:, :, off:off + 4 * (gsz - 1) + 1:4], src)

        # generations: b' 0:12 -> bank cols 0:240, 12:24 -> 240:480,
        # 24:32 -> 0:160 (after gen-A evacuation).  Per-q PSUM banks keep
        # concurrent row-tiles out of each other's banks.  Phase order
        # (4x even across q, then 4x odd) keeps every LDWEIGHTS on a row
        # group with no in-flight matmul, so tiles stream concurrently.
        for bp in range(32):
            gen = bp // 12
            col = 20 * (bp % 12) + (240 if gen == 1 else 0)
            if bp == 12:
                for q2 in range(4):
                    evac(q2, 0, 12, 0)
            elif bp == 24:
                for q2 in range(4):
                    evac(q2, 12, 12, 12)
            for q in range(4):
                lhsT = xt[32 * q:32 * q + 32, bp:bp + 32 * (F - 1) + 1:32]
                rhsE = v_t[32 * q:32 * q + 32, bp:bp + 64 * (KP - 1) + 1:64]
                nc.tensor.matmul(g_ps[q][0:F, col:col + KP], lhsT, rhsE,
                                 start=True, stop=True,
                                 tile_position=(32 * q, 0))
            for q in range(4):
                lhsT = xt[32 * q:32 * q + 32, bp:bp + 32 * (F - 1) + 1:32]
                rhsO = v_t[32 * q:32 * q + 32,
                           32 + bp:32 + bp + 64 * (KP - 1) + 1:64]
                nc.tensor.matmul(g_ps[q][64:64 + F, col:col + KP], lhsT, rhsO,
                                 start=True, stop=True,
                                 tile_position=(32 * q, 64))
        for q2 in range(4):
            evac(q2, 24, 8, 0)

    def emit_final(c):
        ob = out_sb[0:OUT, c * BCH:(c + 1) * BCH]
        if stages < 3:
            nc.vector.memset(ob, 0.0)
            nc.sync.dma_start(out_ext[:, c * BCH:(c + 1) * BCH], ob)
            return
        s_sb = s_chunks[c]
        acc = psum_a.tile([128, BCH], f32, name="acc", tag="acc")
        for k in range(KP):
            nc.tensor.matmul(acc[0:OUT, :], wcp[0:104, k * OUT:(k + 1) * OUT],
                             s_sb[0:104, k * BCH:(k + 1) * BCH],
                             start=(k == 0), stop=(k == KP - 1),
                             tile_position=(0, 0), skip_group_check=True)
        if bc_is_zero:
            copy(ob, acc[0:OUT, :])
        else:
            nc.vector.tensor_scalar_add(ob, acc[0:OUT, :], bc_sb[0:OUT, 0:1])
        nc.sync.dma_start(out_ext[:, c * BCH:(c + 1) * BCH], ob)

    # ---- software pipeline: step1 runs one tile ahead of pairwise so the
    # v evacuation (DVE/ACT) hides behind the previous tile's PE work.
    emit_dma(0)
    s_chunks[0] = spool.tile([128, KP * BCH], bf16, name="s_sb", tag="s_sb")
    emit_step1(0)
    for t in range(nt):
        if t % CHUNK_T == 0 and t // CHUNK_T + 1 < n_chunks:
            emit_dma(t // CHUNK_T + 1)
        if t + 1 < nt:
            if (t + 1) % CHUNK_T == 0:
                s_chunks[t // CHUNK_T + 1] = spool.tile(
                    [128, KP * BCH], bf16, name="s_sb", tag="s_sb")
            emit_step1(t + 1)
        emit_pairwise(t)
        if t % CHUNK_T == CHUNK_T - 1:
            emit_final(t // CHUNK_T)


def _host_prep(fields, W, bw, Wc, bc):
    # x_t[core][32q+d, 1280t + 32f + b'] = fields[f, 2048c + 128t + 4b' + q, d]
    # (in-tile example index 4b'+q == the kernel's pairwise slot order, so
    # S columns / output columns come out in natural batch order)
    x = np.asarray(fields, np.float32).reshape(F, N_CORES, NT, 32, 4, D)
    x_t = np.transpose(x, (1, 4, 5, 2, 0, 3))  # [c, q, d, t, f, b']
    x_t = np.ascontiguousarray(x_t).reshape(N_CORES, 128, NT * TILE_C)
    x_t = x_t.astype(ml_dtypes.bfloat16)

    w4 = np.zeros((128, 128), np.float32)
    for a in range(4):
        w4[32 * a:32 * a + 32, 32 * a:32 * a + 32] = np.asarray(W, np.float32)
    w4 = w4.astype(ml_dtypes.bfloat16)

    iu, ju = np.triu_indices(F, k=1)
    wc_full = np.zeros((F, F, OUT), np.float32)
    wc_full[iu, ju] = np.asarray(Wc, np.float32)          # [i, j, o]
    wcp = np.zeros((128, KP * OUT), np.float32)
    for a in range(2):
        for k in range(KP):
            # rows 64a + j  <-  Wc[i=2k+a, j, :]
            wcp[64 * a:64 * a + F, OUT * k:OUT * (k + 1)] = wc_full[2 * k + a]
    wcp = wcp.astype(ml_dtypes.bfloat16)

    bw_host = np.tile(np.asarray(bw, np.float32).reshape(-1, 1), (4, 1))
    bc_host = np.zeros((128, 1), np.float32)
    bc_host[0:OUT, 0] = np.asarray(bc, np.float32)
    return x_t, w4, wcp, bw_host, bc_host


_WAIT_CAPS = {}
_WAIT_CAP_DEFAULT = 1


def legalize_waits(nc):
    """Walrus codegen accepts only a limited number of sync-wait commands per
    instruction (1 for matmul's S3_LW path, 2 for most others).  Hoist excess
    waits onto no-op instructions on the same engine immediately before."""
    for fn in nc.m.functions:
        for blk in fn.blocks:
            out = []
            for inst in blk.instructions:
                si = getattr(inst, "sync_info", None)
                waits = list(si.on_wait) if si is not None and si.on_wait else []
                cap = _WAIT_CAPS.get(type(inst).__name__, _WAIT_CAP_DEFAULT)
                if len(waits) > cap:
                    excess, keep = waits[:-cap], waits[-cap:]
                    for ci, w in enumerate(excess):
                        nop = mybir.InstNoOp(
                            name=f"{inst.name}-waitsplit{ci}",
                            sync_info=mybir.SyncInfo(on_wait=[w], on_update=[]),
                            bass_nofuse=True,
                            engine=inst.engine,
                        )
                        out.append(nop)
                    si.on_wait = keep
                out.append(inst)
            blk.instructions[:] = out
    return nc


def make_nc(bc_count=BC, bw_is_zero=False, bc_is_zero=False, legalize=True,
            stages=3):
    nc = bass.Bass()
    bf16 = mybir.dt.bfloat16
    x_ext = nc.declare_dram_parameter("x_t", [128, (bc_count // 128) * TILE_C],
                                      bf16, isOutput=False)
    w4_ext = nc.declare_dram_parameter("w4", [128, 128], bf16, isOutput=False)
    wcp_ext = nc.declare_dram_parameter("wcp", [128, KP * OUT], bf16,
                                        isOutput=False)
    bw_ext = nc.declare_dram_parameter("bw_r", [128, 1], mybir.dt.float32,
                                       isOutput=False)
    bc_ext = nc.declare_dram_parameter("bc_r", [128, 1], mybir.dt.float32,
                                       isOutput=False)
    out_ext = nc.declare_dram_parameter("out_t", [OUT, bc_count],
                                        mybir.dt.float32, isOutput=True)
    with tile.TileContext(nc) as tc:
        build_kernel(tc, out_ext, x_ext, w4_ext, wcp_ext, bw_ext, bc_ext,
                     bc_count=bc_count, bw_is_zero=bw_is_zero,
                     bc_is_zero=bc_is_zero, stages=stages)
    if legalize:
        legalize_waits(nc)
    return nc


def kernel(fields, W, bw, Wc, bc):
    x_t, w4, wcp, bw_host, bc_host = _host_prep(fields, W, bw, Wc, bc)
    nc = make_nc(BC, bw_is_zero=bool(np.all(np.asarray(bw) == 0)),
                 bc_is_zero=bool(np.all(np.asarray(bc) == 0)))
    in_maps = []
    for c in range(N_CORES):
        in_maps.append({
            "x_t": np.ascontiguousarray(x_t[c]),
            "w4": w4, "wcp": wcp, "bw_r": bw_host, "bc_r": bc_host,
        })
    res = run_bass_kernel_spmd(nc, in_maps, list(range(N_CORES)))
    outs = [res.results[c]["out_t"] for c in range(N_CORES)]  # [50, 2048] each
    full = np.concatenate(outs, axis=1)                        # [50, 16384]
    return np.ascontiguousarray(full.T).astype(np.float32)


# revision 19
# speedup vs baseline: 5.2028x; 1.2195x over previous
"""Trainium2 Bass kernel for a bilinear field-interaction layer.

Computation (per example b):
  v[f]   = fields[f, b] @ W + bw                      # shared Dense(D)
  p[i,j] = dot(v[i], fields[j, b])  for i < j          # 780 pairs
  out[b] = p @ Wc + bc                                 # Dense(OUT)

Shapes: fields [40, 16384, 32], W [32, 32], Wc [780, 50] -> out [16384, 50].

Strategy (8 NeuronCores, batch-sharded 2048/core, all-bf16 compute):
  - HOST pre-transposes fields into the on-chip layout x_t[32q+d,
    1280t + 32f + b'] (bf16), so DMA is one contiguous 10KB-run/partition
    load per 4-tile chunk and no on-device transpose exists.
  - step1: v = x W as ONE matmul per 512-col PSUM segment using a
    host-built block-diagonal W4 [128,128] (4 replicas of W on the
    diagonal) -> v_t bf16, same (32i+b') column layout as x_t.
  - pairwise, per example (q, b'): stationary X^T [32d, 40j] (col stride
    32), two N=20 matmuls moving v-even / v-odd i columns (stride 64)
    into PSUM partition rows 0:40 / 64:104.  G[j, i] = v_i . x_j.
  - G banks of 24 examples evacuated (DVE/ACT alternating) to
    S[64a + j, 512k + b] bf16 where i = 2k + a  (i-pairs along K).
  - final: per 512-example chunk, 20 accumulating matmuls
    acc[o, b] += WCP[:, k]^T @ S[:, k-slice]  with K=104 (j rows 40:64
    are PSUM garbage x host-zeroed Wc rows -> 0).  Output stays [o, b];
    host transposes back.  bias adds fused on the PSUM evacuations.
"""

import sys

sys.path.insert(0, "/opt/trn_rl_repo")

from contextlib import ExitStack

import numpy as np
import ml_dtypes

import concourse.bass as bass
import concourse.tile as tile
from concourse import mybir
from concourse._compat import with_exitstack
from concourse.bass_utils import run_bass_kernel_spmd

F, D, OUT = 40, 32, 50
N_CORES = 8
B_FULL = 16384
BC = B_FULL // N_CORES   # 2048 per core
NT = BC // 128           # 16 b-tiles per core
CHUNK_T = 4              # b-tiles per chunk (S/final granularity)
BCH = CHUNK_T * 128      # 512 examples per chunk
TILE_C = F * D           # 1280 bf16 cols per b-tile
KP = F // 2              # 20 i-pairs (i = 2k + a)
NG = 4                   # rotating G PSUM tiles


@with_exitstack
def build_kernel(ctx: ExitStack, tc: tile.TileContext, out_ext, x_ext,
                 w4_ext, wcp_ext, bw_ext, bc_ext, bc_count=BC,
                 bw_is_zero=False, bc_is_zero=False, stages=3):
    nc = tc.nc
    f32 = mybir.dt.float32
    bf16 = mybir.dt.bfloat16
    nt = bc_count // 128
    n_chunks = nt // CHUNK_T

    const = ctx.enter_context(tc.tile_pool(name="const", bufs=1))
    xpool = ctx.enter_context(tc.tile_pool(name="xpool", bufs=2))
    vpool = ctx.enter_context(tc.tile_pool(name="vpool", bufs=2))
    spool = ctx.enter_context(tc.tile_pool(name="spool", bufs=2))
    opool = ctx.enter_context(tc.tile_pool(name="opool", bufs=1))
    psum_v = ctx.enter_context(tc.tile_pool(name="psv", bufs=2, space="PSUM"))
    psum_g = ctx.enter_context(tc.tile_pool(name="psg", bufs=1, space="PSUM"))
    psum_a = ctx.enter_context(tc.tile_pool(name="psa", bufs=2, space="PSUM"))

    # ---- constants
    w4 = const.tile([128, 128], bf16)
    nc.sync.dma_start(w4[:], w4_ext[:])
    wcp = const.tile([128, KP * OUT], bf16)
    nc.sync.dma_start(wcp[:], wcp_ext[:])
    bw_sb = const.tile([128, 1], f32)
    nc.sync.dma_start(bw_sb[:], bw_ext[:])
    bc_sb = const.tile([128, 1], f32)
    nc.sync.dma_start(bc_sb[:], bc_ext[:])

    out_sb = opool.tile([128, bc_count], f32)

    # ---- block-diagonal v buffers: vbd[32q+d, 160bp + 40q + i] holds
    # v[i, ex(q,bp), d]; off-diagonal stays zero forever (memset once).
    v_bd = [opool.tile([128, 32 * 160], bf16, name=f"vbd{i}") for i in range(2)]
    nc.vector.memset(v_bd[0][:], 0.0)
    nc.gpsimd.memset(v_bd[1][:], 0.0)

    # ---- persistent pairwise PSUM banks; rows 40:64 are read by the S
    # evacuation (as K padding vs zeroed Wc rows) -> must be finite.
    g_ps = [psum_g.tile([128, 320], f32, name=f"gps{i}") for i in range(NG)]
    for i in range(NG):
        nc.vector.memset(g_ps[i][32:64, :], 0.0)

    copy_fns = [
        lambda d, s: nc.vector.tensor_copy(d, s),
        lambda d, s: nc.scalar.copy(d, s),
    ]
    cp_idx = [0]

    def copy(dst, src):
        copy_fns[cp_idx[0] % 2](dst, src)
        cp_idx[0] += 1

    x_chunks = [None] * n_chunks
    s_chunks = [None] * n_chunks
    v_tiles = [None] * nt

    def emit_dma(c):
        x_sb = xpool.tile([128, CHUNK_T * TILE_C], bf16, name="x_sb",
                          tag="x_sb")
        nc.sync.dma_start(x_sb[:], x_ext[:, c * CHUNK_T * TILE_C:
                                         (c + 1) * CHUNK_T * TILE_C])
        x_chunks[c] = x_sb

    def emit_step1(t):
        c, tt = divmod(t, CHUNK_T)
        xt = x_chunks[c][:, tt * TILE_C:(tt + 1) * TILE_C]
        v_t = vpool.tile([128, TILE_C], bf16, name="v_t", tag="v_t")
        v_tiles[t] = v_t
        for n0, n1 in ((0, 512), (512, 1024), (1024, TILE_C)):
            v_ps = psum_v.tile([128, 512], f32, name="v_ps", tag="v_ps")
            nc.tensor.matmul(v_ps[:, :n1 - n0], w4[:], xt[:, n0:n1],
                             start=True, stop=True, tile_position=(0, 0))
            if bw_is_zero:
                copy(v_t[:, n0:n1], v_ps[:, :n1 - n0])
            else:
                nc.vector.tensor_scalar_add(v_t[:, n0:n1], v_ps[:, :n1 - n0],
                                            bw_sb[:, 0:1])

    def emit_scatter(t):
        # v_t[32q+d, 32i+bp] -> v_bd[32q+d, 1280q + 32i + bp] via four
        # contiguous SBUF->SBUF DMAs (partition-dependent column base
        # forces the per-q split; DMA engines are otherwise idle).  Rows
        # of other q' in col-block q stay zero (block diagonal).
        vb = v_bd[t % 2]
        v_t = v_tiles[t]
        for q in range(4):
            nc.sync.dma_start(vb[32 * q:32 * q + 32,
                                 1280 * q:1280 * (q + 1)],
                              v_t[32 * q:32 * q + 32, :])

    def emit_pairwise(t):
        if stages < 2:
            return
        c, tt = divmod(t, CHUNK_T)
        xt = x_chunks[c][:, tt * TILE_C:(tt + 1) * TILE_C]
        vb = v_bd[t % 2]
        s_sb = s_chunks[c]
        s_r = s_sb[0:104, :].rearrange("p (k t b q) -> p t b q k",
                                       t=CHUNK_T, b=32, q=4)

        def evac(bp0):
            # bank holding groups bp0..bp0+4 -> S cols BCH*k + 128tt +
            # 4(bp0+g) + q; rows 40:64 carry the zero/garbage K-pad.
            g = g_ps[(bp0 // 4) % NG]
            src = g[0:104, 0:320].rearrange("p (g q k) -> p g q k",
                                            q=4, k=KP)
            copy(s_r[:, tt, bp0:bp0 + 4, :, :], src)

        # K-packed pairwise: 4 examples (q=0..3) of group bp stacked along
        # K=128; lhsT = stacked X (one ldweights per group), rhs = block-
        # diagonal v columns, split even/odd i for the i-pair final layout.
        vr = vb[:, :].rearrange("p (q i b) -> p q i b", q=4, b=32)
        for bp in range(32):
            g = g_ps[(bp // 4) % NG]
            sc = 80 * (bp % 4)
            lhsT = xt[:, bp:bp + 32 * (F - 1) + 1:32]
            vs = vr[:, :, :, bp]                       # [128, q:4, i:40]
            rhs_e = vs[:, :, 0:2 * (KP - 1) + 1:2]
            rhs_o = vs[:, :, 1:2 * (KP - 1) + 2:2]
            nc.tensor.matmul(g[0:F, sc:sc + 80], lhsT, rhs_e,
                             start=True, stop=True, tile_position=(0, 0))
            nc.tensor.matmul(g[64:64 + F, sc:sc + 80], lhsT, rhs_o,
                             start=True, stop=True, tile_position=(0, 64))
            if bp % 4 == 3:
                evac(bp - 3)

    def emit_final(c):
        ob = out_sb[0:OUT, c * BCH:(c + 1) * BCH]
        if stages < 3:
            nc.vector.memset(ob, 0.0)
            nc.sync.dma_start(out_ext[:, c * BCH:(c + 1) * BCH], ob)
            return
        s_sb = s_chunks[c]
        acc = psum_a.tile([128, BCH], f32, name="acc", tag="acc")
        for k in range(KP):
            nc.tensor.matmul(acc[0:OUT, :], wcp[0:104, k * OUT:(k + 1) * OUT],
                             s_sb[0:104, k * BCH:(k + 1) * BCH],
                             start=(k == 0), stop=(k == KP - 1),
                             tile_position=(0, 0), skip_group_check=True)
        if bc_is_zero:
            copy(ob, acc[0:OUT, :])
        else:
            nc.vector.tensor_scalar_add(ob, acc[0:OUT, :], bc_sb[0:OUT, 0:1])
        nc.sync.dma_start(out_ext[:, c * BCH:(c + 1) * BCH], ob)

    # ---- software pipeline: step1 runs one tile ahead of pairwise so the
    # v evacuation (DVE/ACT) hides behind the previous tile's PE work.
    emit_dma(0)
    s_chunks[0] = spool.tile([128, KP * BCH], bf16, name="s_sb", tag="s_sb")
    emit_step1(0)
    emit_scatter(0)
    for t in range(nt):
        if t % CHUNK_T == 0 and t // CHUNK_T + 1 < n_chunks:
            emit_dma(t // CHUNK_T + 1)
        if t + 1 < nt:
            if (t + 1) % CHUNK_T == 0:
                s_chunks[t // CHUNK_T + 1] = spool.tile(
                    [128, KP * BCH], bf16, name="s_sb", tag="s_sb")
            emit_step1(t + 1)
            emit_scatter(t + 1)
        emit_pairwise(t)
        if t % CHUNK_T == CHUNK_T - 1:
            emit_final(t // CHUNK_T)


def _host_prep(fields, W, bw, Wc, bc):
    # x_t[core][32q+d, 1280t + 32f + b'] = fields[f, 2048c + 128t + 4b' + q, d]
    # (in-tile example index 4b'+q == the kernel's pairwise slot order, so
    # S columns / output columns come out in natural batch order)
    x = np.asarray(fields, np.float32).reshape(F, N_CORES, NT, 32, 4, D)
    x_t = np.transpose(x, (1, 4, 5, 2, 0, 3))  # [c, q, d, t, f, b']
    x_t = np.ascontiguousarray(x_t).reshape(N_CORES, 128, NT * TILE_C)
    x_t = x_t.astype(ml_dtypes.bfloat16)

    w4 = np.zeros((128, 128), np.float32)
    for a in range(4):
        w4[32 * a:32 * a + 32, 32 * a:32 * a + 32] = np.asarray(W, np.float32)
    w4 = w4.astype(ml_dtypes.bfloat16)

    iu, ju = np.triu_indices(F, k=1)
    wc_full = np.zeros((F, F, OUT), np.float32)
    wc_full[iu, ju] = np.asarray(Wc, np.float32)          # [i, j, o]
    wcp = np.zeros((128, KP * OUT), np.float32)
    for a in range(2):
        for k in range(KP):
            # rows 64a + j  <-  Wc[i=2k+a, j, :]
            wcp[64 * a:64 * a + F, OUT * k:OUT * (k + 1)] = wc_full[2 * k + a]
    wcp = wcp.astype(ml_dtypes.bfloat16)

    bw_host = np.tile(np.asarray(bw, np.float32).reshape(-1, 1), (4, 1))
    bc_host = np.zeros((128, 1), np.float32)
    bc_host[0:OUT, 0] = np.asarray(bc, np.float32)
    return x_t, w4, wcp, bw_host, bc_host


_WAIT_CAPS = {}
_WAIT_CAP_DEFAULT = 1


def legalize_waits(nc):
    """Walrus codegen accepts only a limited number of sync-wait commands per
    instruction (1 for matmul's S3_LW path, 2 for most others).  Hoist excess
    waits onto no-op instructions on the same engine immediately before."""
    for fn in nc.m.functions:
        for blk in fn.blocks:
            out = []
            for inst in blk.instructions:
                si = getattr(inst, "sync_info", None)
                waits = list(si.on_wait) if si is not None and si.on_wait else []
                cap = _WAIT_CAPS.get(type(inst).__name__, _WAIT_CAP_DEFAULT)
                if len(waits) > cap:
                    excess, keep = waits[:-cap], waits[-cap:]
                    for ci, w in enumerate(excess):
                        nop = mybir.InstNoOp(
                            name=f"{inst.name}-waitsplit{ci}",
                            sync_info=mybir.SyncInfo(on_wait=[w], on_update=[]),
                            bass_nofuse=True,
                            engine=inst.engine,
                        )
                        out.append(nop)
                    si.on_wait = keep
                out.append(inst)
            blk.instructions[:] = out
    return nc


def make_nc(bc_count=BC, bw_is_zero=False, bc_is_zero=False, legalize=True,
            stages=3):
    nc = bass.Bass()
    bf16 = mybir.dt.bfloat16
    x_ext = nc.declare_dram_parameter("x_t", [128, (bc_count // 128) * TILE_C],
                                      bf16, isOutput=False)
    w4_ext = nc.declare_dram_parameter("w4", [128, 128], bf16, isOutput=False)
    wcp_ext = nc.declare_dram_parameter("wcp", [128, KP * OUT], bf16,
                                        isOutput=False)
    bw_ext = nc.declare_dram_parameter("bw_r", [128, 1], mybir.dt.float32,
                                       isOutput=False)
    bc_ext = nc.declare_dram_parameter("bc_r", [128, 1], mybir.dt.float32,
                                       isOutput=False)
    out_ext = nc.declare_dram_parameter("out_t", [OUT, bc_count],
                                        mybir.dt.float32, isOutput=True)
    with tile.TileContext(nc) as tc:
        build_kernel(tc, out_ext, x_ext, w4_ext, wcp_ext, bw_ext, bc_ext,
                     bc_count=bc_count, bw_is_zero=bw_is_zero,
                     bc_is_zero=bc_is_zero, stages=stages)
    if legalize:
        legalize_waits(nc)
    return nc


def kernel(fields, W, bw, Wc, bc):
    x_t, w4, wcp, bw_host, bc_host = _host_prep(fields, W, bw, Wc, bc)
    nc = make_nc(BC, bw_is_zero=bool(np.all(np.asarray(bw) == 0)),
                 bc_is_zero=bool(np.all(np.asarray(bc) == 0)))
    in_maps = []
    for c in range(N_CORES):
        in_maps.append({
            "x_t": np.ascontiguousarray(x_t[c]),
            "w4": w4, "wcp": wcp, "bw_r": bw_host, "bc_r": bc_host,
        })
    res = run_bass_kernel_spmd(nc, in_maps, list(range(N_CORES)))
    outs = [res.results[c]["out_t"] for c in range(N_CORES)]  # [50, 2048] each
    full = np.concatenate(outs, axis=1)                        # [50, 16384]
    return np.ascontiguousarray(full.T).astype(np.float32)
